# revision 1
# baseline (speedup 1.0000x reference)
"""Trainium2 Bass kernel for nn_Classification_4922032521468.

Problem: acts = embeds[activity_index]  (A=512 rows, d=512)
         pairs = concat(acts[ii], acts[jj])  for all i<j (P=130816 pairs)
         out = log_softmax(pairs @ W.T + b)  -> [P, 4]

Key algebra: logits[p, c] = L[i, c] + R'[j, c]  with
  L  = acts @ Wl.T          (Wl = W[:, :512])
  R' = acts @ Wr.T + b      (Wr = W[:, 512:])
so log_softmax needs only lse[i, j] = ln(sum_c e^{L[i,c]} e^{R'[j,c]})
(a K=4 PE matmul of U = e^L rows against V = e^{R'}) and
  out[i, j, c] = L[i, c] + R'[j, c] - lse[i, j].
No 130816x1024 pair tensor is ever built.

Layout: the per-core output plane is computed TRANSPOSED - j on partitions,
(i, c) on the free axis - which makes every term either per-partition
(R', lse) or a partition-broadcast row (L, built once with a K=1 matmul).

Sharding: core k owns i-rows [64k, 64k+64). The same NEFF runs on all 8
cores (SPMD); per-core behavior comes only from per-core DATA:
activity_index is rotated by -64k so each core's own i-rows are gathered
rows 0..63. Each core outputs [512 j, 64 i, 4 c] (j rotated); the host
un-rotates j, transposes, and gathers the triu pairs.
"""

import numpy as np

A = 512  # number of activity tokens
D = 512  # embedding dim
C = 4  # classes
NTOK = 4096  # embeds table rows
RB = 64  # i-rows per core
NCORES = 8

_program = None
_last_results = None  # BassKernelResults from the most recent run (profiling)


def _build_program():
    from contextlib import ExitStack

    import concourse.bacc as bacc
    import concourse.mybir as mybir
    import concourse.tile as tile
    from concourse.bass import IndirectOffsetOnAxis
    from concourse.tile_rust import add_dep_helper

    fp32 = mybir.dt.float32
    i32 = mybir.dt.int32
    AF = mybir.ActivationFunctionType
    SUB = mybir.AluOpType.subtract
    ADD = mybir.AluOpType.add

    nc = bacc.Bacc(
        "TRN2",
        target_bir_lowering=False,
        debug=False,
        enable_asserts=False,
        num_devices=NCORES,
    )

    embeds_h = nc.dram_tensor("embeds", (NTOK, D), fp32, kind="ExternalInput")
    # idxs[p, j] = rotated activity_index[128j + p], int32
    idx_h = nc.dram_tensor("idxs", (128, 4), i32, kind="ExternalInput")
    # wt[d, 8k+0:4] = Wr.T[128k+d, :], wt[d, 8k+4:8] = Wl.T[128k+d, :]
    wt_h = nc.dram_tensor("wt", (128, 32), fp32, kind="ExternalInput")
    # b8 = [b_0..b_3, 0, 0, 0, 0] (bias folds into R via a K=1 matmul)
    b8_h = nc.dram_tensor("b8", (1, 8), fp32, kind="ExternalInput")
    # out[j, 4i + c] (j rotated per core)
    out_h = nc.dram_tensor("out", (A, RB * C), fp32, kind="ExternalOutput")

    ident_h = nc.inline_tensor(np.eye(128, dtype=np.float32), name="ident")

    embeds_ap = embeds_h.ap()
    out_ap = out_h.ap()

    with tile.TileContext(nc) as tc, ExitStack() as ctx:
        sb = ctx.enter_context(tc.tile_pool(name="sb", bufs=1))
        sbr = ctx.enter_context(tc.tile_pool(name="sbr", bufs=6))
        psT = ctx.enter_context(tc.tile_pool(name="psT", bufs=3, space="PSUM"))
        psR = ctx.enter_context(tc.tile_pool(name="psR", bufs=2, space="PSUM"))
        psB = ctx.enter_context(tc.tile_pool(name="psB", bufs=1, space="PSUM"))
        psS = ctx.enter_context(tc.tile_pool(name="psS", bufs=1, space="PSUM"))

        # ---- gather path first: idx load, then the 4 indirect gathers ----
        idxs = sb.tile([128, 4], i32, tag="idxs")
        nc.sync.dma_start(out=idxs[:], in_=idx_h.ap()[:])

        acts = []
        for j in range(4):
            aj = sb.tile([128, D], fp32, tag=f"acts{j}", name=f"acts{j}")
            nc.gpsimd.indirect_dma_start(
                out=aj[:],
                out_offset=None,
                in_=embeds_ap[:],
                in_offset=IndirectOffsetOnAxis(ap=idxs[:, j : j + 1], axis=0),
            )
            acts.append(aj)

        # ---- small constants (dispatch behind idx on the sync queue) ----
        ident = sb.tile([128, 128], fp32, tag="ident")
        nc.sync.dma_start(out=ident[:], in_=ident_h.ap()[:])
        wt = sb.tile([128, 32], fp32, tag="wt")
        nc.sync.dma_start(out=wt[:], in_=wt_h.ap()[:])
        b4 = sb.tile([C, 1], fp32, tag="b4")
        nc.sync.dma_start(out=b4[:], in_=b8_h.ap()[0:1, 0:C])
        ones = sb.tile([1, 128], fp32, tag="ones")
        nc.vector.memset(ones[:], 1.0)

        # persistent tiles
        rj = sb.tile([128, 16], fp32, tag="rj")  # R' row-major, chunk j cols 4j:4j+4
        rt = sb.tile([C, A], fp32, tag="rt")  # R' transposed
        vt = sb.tile([C, A], fp32, tag="vt")  # e^{R'} transposed (classes on K)
        ut4 = sb.tile([C, RB], fp32, tag="ut4")  # e^{L} transposed
        lt4 = sb.tile([C, RB], fp32, tag="lt4")  # L transposed
        lbf = sb.tile([1, RB * C], fp32, tag="lbf")  # L flattened (4i + c)

        # ---- phase A per j-chunk: transpose, R' matmuls, e^{R'} ----
        # (all Exp ops are emitted before any Ln so the ACT table loads once
        # per function instead of thrashing Exp<->Ln. Matmuls keep the tiny
        # wt as the STATIONARY operand - a [128, 128] stationary would pay a
        # ~1.3us weight load per call.)
        for j in range(4):
            aT = []
            for k in range(4):
                pt = psT.tile([128, 128], fp32, tag="pt", name="pt")
                nc.tensor.transpose(
                    out=pt[:],
                    in_=acts[j][:, 128 * k : 128 * k + 128],
                    identity=ident[:],
                )
                at = sbr.tile([128, 128], fp32, tag="aT", name="aT")
                nc.vector.tensor_copy(out=at[:], in_=pt[:])
                aT.append(at)

            # R'^T chunk [4, 128] = sum_k Wr.T_k.T @ aT_k  (+ b outer ones)
            pr = psR.tile([C, 128], fp32, tag="pr", name="pr")
            for k in range(4):
                nc.tensor.matmul(
                    out=pr[:],
                    lhsT=wt[:, 8 * k : 8 * k + 4],
                    rhs=aT[k][:],
                    start=(k == 0),
                    stop=(k == 3),
                )
            # b rides for free: ACT bias on the exp, DVE scalar-add on rt
            # (classes sit on partitions here, so b is a [4, 1] per-partition
            # operand) - no K=1 PE matmul needed.
            nc.vector.tensor_scalar_add(
                rt[:, 128 * j : 128 * (j + 1)], pr[:], b4[:]
            )
            last_exp = nc.scalar.activation(
                out=vt[:, 128 * j : 128 * (j + 1)],
                in_=pr[:],
                func=AF.Exp,
                bias=b4[:],
            )
            # row-major chunk for the final per-partition add
            prj = psT.tile([128, C], fp32, tag="pt", name="prj")
            nc.tensor.transpose(
                out=prj[:],
                in_=rt[:, 128 * j : 128 * (j + 1)],
                identity=ident[0:C, 0:C],
            )
            nc.vector.tensor_copy(out=rj[:, 4 * j : 4 * j + 4], in_=prj[:])

            if j == 0:
                # L^T [4, 64] (no bias; b lives on the R side)
                pl = psR.tile([C, RB], fp32, tag="pl", name="pl", bufs=1)
                for k in range(4):
                    nc.tensor.matmul(
                        out=pl[:],
                        lhsT=wt[:, 8 * k + 4 : 8 * k + 8],
                        rhs=aT[k][:, 0:RB],
                        start=(k == 0),
                        stop=(k == 3),
                    )
                nc.scalar.activation(out=ut4[:], in_=pl[:], func=AF.Exp)
                nc.vector.tensor_copy(out=lt4[:], in_=pl[:])
                # lbf[0, 4i+c] = L[i, c] via per-class reordering DMAs
                lbf3 = lbf[:].rearrange("o (i c) -> o i c", c=C)
                for c in range(C):
                    nc.sync.dma_start(
                        out=lbf3[:, :, c : c + 1], in_=lt4[c : c + 1, :]
                    )

        # L broadcast across all 128 partitions via K=1 matmul (kept in PSUM)
        lbb = psB.tile([128, RB * C], fp32, tag="lbb")
        nc.tensor.matmul(out=lbb[:], lhsT=ones[:], rhs=lbf[:], start=True, stop=True)
        lbb3 = lbb[:].rearrange("p (i c) -> p i c", c=C)

        # ---- phase B per j-chunk: lse, combine, store ----
        for j in range(4):
            se = psS.tile([128, RB], fp32, tag="se", name="se")
            nc.tensor.matmul(
                out=se[:],
                lhsT=vt[:, 128 * j : 128 * (j + 1)],
                rhs=ut4[:],
                start=True,
                stop=True,
            )
            lnse = sbr.tile([128, RB], fp32, tag="lnse", name="lnse")
            ln_inst = nc.scalar.activation(out=lnse[:], in_=se[:], func=AF.Ln)
            # keep every Ln after the last Exp so the ACT function table
            # loads exactly twice instead of thrashing Exp<->Ln per chunk
            add_dep_helper(
                ln_inst.ins, last_exp.ins, sync=False, reason="act-table order"
            )

            tmp = sbr.tile([128, RB * C], fp32, tag="tmp", name="tmp")
            nc.vector.tensor_tensor(
                out=tmp[:].rearrange("p (i c) -> p i c", c=C),
                in0=lbb3,
                in1=lnse[:].unsqueeze(2).to_broadcast([128, RB, C]),
                op=SUB,
            )
            oj = sbr.tile([128, RB * C], fp32, tag="oj", name="oj")
            nc.vector.tensor_tensor(
                out=oj[:].rearrange("p (i c) -> p i c", c=C),
                in0=tmp[:].rearrange("p (i c) -> p i c", c=C),
                in1=rj[:, 4 * j : 4 * j + 4].unsqueeze(1).to_broadcast([128, RB, C]),
                op=ADD,
            )
            nc.sync.dma_start(
                out=out_ap[128 * j : 128 * (j + 1), :], in_=oj[:]
            )

    nc.compile()
    return nc


def _get_program():
    global _program
    if _program is None:
        _program = _build_program()
    return _program


def _prep_core_inputs(embeds, idx64, wt_np, b8_np, k):
    rot = np.roll(idx64, -RB * k)
    idxs = np.ascontiguousarray(rot.reshape(4, 128).T.astype(np.int32))
    return {"embeds": embeds, "idxs": idxs, "wt": wt_np, "b8": b8_np}


def kernel(embeds, activity_index, W, b):
    from concourse.bass_utils import run_bass_kernel_spmd

    embeds = np.ascontiguousarray(np.asarray(embeds), dtype=np.float32)
    W = np.asarray(W, dtype=np.float32)
    b_in = np.asarray(b, dtype=np.float32).reshape(C)
    idx64 = np.asarray(activity_index).astype(np.int64)

    # wt[d, 8k+0:4] = Wr.T chunk k, wt[d, 8k+4:8] = Wl.T chunk k
    wt_np = np.empty((128, 32), dtype=np.float32)
    for k in range(4):
        wt_np[:, 8 * k : 8 * k + 4] = W[:, D + 128 * k : D + 128 * (k + 1)].T
        wt_np[:, 8 * k + 4 : 8 * k + 8] = W[:, 128 * k : 128 * (k + 1)].T
    wt_np = np.ascontiguousarray(wt_np)
    b8_np = np.zeros((1, 8), dtype=np.float32)
    b8_np[0, 0:C] = b_in

    nc = _get_program()
    in_maps = [
        _prep_core_inputs(embeds, idx64, wt_np, b8_np, k) for k in range(NCORES)
    ]

    results = run_bass_kernel_spmd(nc, in_maps, core_ids=list(range(NCORES)))
    global _last_results
    _last_results = results

    out_sq = np.empty((A, A, C), dtype=np.float32)
    for k in range(NCORES):
        # blk[j, i, c] with j rotated by -64k -> un-rotate and transpose
        blk = results.results[k]["out"].reshape(A, RB, C).transpose(1, 0, 2)
        out_sq[RB * k : RB * (k + 1)] = np.roll(blk, RB * k, axis=1)

    ii, jj = np.triu_indices(A, k=1)
    return np.ascontiguousarray(out_sq[ii, jj])

